# revision 1
# baseline (speedup 1.0000x reference)
"""Trainium2 Bass kernel: nn_AttentionLayer (T=2048, B=2, H=16, N_in=1024, d=64).

Sharding: head-parallel across 8 NeuronCores. Each core gets the full x plus a
128-row slice (2 heads) of Wk/Wq/Wv and biases, computes attention for its 2
heads x 2 batches, and writes out[:, :, c*128:(c+1)*128]. The host concatenates
the 8 shards along the feature axis. No cross-core collectives.

Per-core math (matching the reference):
  K^T, Q^T, V^T = W_slice @ x_b^T + bias          (out^T layout, [128, T])
  S^T[k, i]     = sum_n Q^T[n,k] * K^T[n,i]        (k = softmax/query axis)
  E             = exp(S^T / 32)                    (bf16, safe range: |S/32| < ~1.5)
  out'[i, 0:64] = sum_k E[k,i] * V[k, :]           (A@V numerator)
  out'[i, 64]   = sum_k E[k,i]                     (softmax denominator via a
                                                    ones-column appended to V)
  out[i, :]     = out'[i, 0:64] / out'[i, 64]

x^T is produced by a f32->bf16 cast DMA (SWDGE) into a DRAM bounce followed by
DMA-xbar transposes (HWDGE); W^T and V are produced with PE matmul-by-identity
transposes. All matmuls run in bf16 (fp32 matmul is 2x slower on the PE).
"""

import numpy as np

T = 2048
B = 2
NIN = 1024
NQK = 64
NCORES = 8
H_PER_CORE = 2
GD = H_PER_CORE * NQK  # 128: per-core projection width (2 heads x 64)

NT = NIN // 128  # 8  n-tiles (contraction tiles for projections)
TT = T // 128    # 16 t-tiles
ICH = 2          # i-chunks per (b, h)
IC_LEN = T // ICH
IT = IC_LEN // 128  # 8 i-tiles per chunk
JT = TT          # 16 k-tiles (softmax axis)

_CACHE = {}


def _build():
    import concourse.bass as bass
    import concourse.tile as tile
    from concourse import bacc, mybir
    from concourse.masks import make_identity

    f32 = mybir.dt.float32
    bf16 = mybir.dt.bfloat16
    AF = mybir.ActivationFunctionType

    nc = bacc.Bacc("TRN2", target_bir_lowering=False, debug=False,
                   num_devices=NCORES)

    x = nc.dram_tensor("x", [T, B, NIN], f32, kind="ExternalInput").ap()
    w_in = {
        "k": nc.dram_tensor("wk", [GD, NIN], f32, kind="ExternalInput").ap(),
        "q": nc.dram_tensor("wq", [GD, NIN], f32, kind="ExternalInput").ap(),
        "v": nc.dram_tensor("wv", [GD, NIN], f32, kind="ExternalInput").ap(),
    }
    b_in = {
        "k": nc.dram_tensor("bk", [GD], f32, kind="ExternalInput").ap(),
        "q": nc.dram_tensor("bq", [GD], f32, kind="ExternalInput").ap(),
        "v": nc.dram_tensor("bv", [GD], f32, kind="ExternalInput").ap(),
    }
    out = nc.dram_tensor("out", [T, B, GD], f32, kind="ExternalOutput").ap()

    with tile.TileContext(nc) as tc:
        with (
            tc.tile_pool(name="const", bufs=1) as const_pool,
            tc.tile_pool(name="wstage", bufs=2) as wstage,
            tc.tile_pool(name="wt", bufs=1) as wt_pool,
            tc.tile_pool(name="xt", bufs=2) as xt_pool,
            tc.tile_pool(name="pt", bufs=1) as pt_pool,
            tc.tile_pool(name="vp", bufs=1) as vp_pool,
            tc.tile_pool(name="es", bufs=18) as es_pool,
            tc.tile_pool(name="of", bufs=3) as of_pool,
            tc.tile_pool(name="sm", bufs=4) as sm_pool,
            tc.tile_pool(name="dram", bufs=1, space="DRAM") as dram_pool,
            tc.tile_pool(name="ps_s", bufs=2, space="PSUM") as ps_s,
            tc.tile_pool(name="ps_av", bufs=2, space="PSUM") as ps_av,
        ):
            # --- constants -------------------------------------------------
            ident = const_pool.tile([128, 128], f32, name="ident")
            make_identity(nc, ident)
            identb = const_pool.tile([128, 128], bf16, name="identb")
            nc.vector.tensor_copy(out=identb[:], in_=ident[:])

            bias_t = {}
            for p in ("k", "q", "v"):
                bt = const_pool.tile([128, 1], f32, name=f"bias_{p}")
                nc.sync.dma_start(out=bt[:], in_=b_in[p].rearrange("(p o) -> p o", o=1))
                bias_t[p] = bt

            # --- weights: W [128, 1024] -> W^T tiles [128(n), nt, 128(g)] bf16
            wt = {}
            for p in ("k", "q", "v"):
                wnat = wstage.tile([128, NIN], f32, name=f"wnat_{p}", tag="wnat")
                nc.sync.dma_start(out=wnat[:], in_=w_in[p])
                wps = ps_av.tile([128, 2, 512], f32, name=f"wps_{p}", tag="av")
                for nt in range(NT):
                    nc.tensor.matmul(
                        wps[:, nt // 4, (nt % 4) * 128:(nt % 4) * 128 + 128],
                        lhsT=wnat[:, nt * 128:(nt + 1) * 128],
                        rhs=ident[:],
                        start=True, stop=True,
                    )
                w_t = wt_pool.tile([128, NT, 128], bf16, name=f"wt_{p}", tag=f"wt_{p}")
                nc.vector.tensor_copy(
                    out=w_t[:],
                    in_=wps.rearrange("p a (c m) -> p (a c) m", m=128),
                )
                wt[p] = w_t

            # --- per-batch: x^T, projections, V' --------------------------
            pt = {}   # pt[(p, b)]: [128(g), T] bf16   (g = 2 heads x 64)
            vp = {}   # vp[(h, b)]: [128(t), JT, 65] bf16 (V plus ones column)

            def prep_batch(b):
                xbf = dram_pool.tile([T, NIN], bf16, name=f"xbf_{b}", tag=f"xbf_{b}")
                nc.gpsimd.dma_start(out=xbf[:], in_=x[:, b, :])  # f32->bf16 cast
                xT = xt_pool.tile([128, NT, T], bf16, name=f"xT_{b}", tag="xT")
                for nt in range(NT):
                    nc.sync.dma_start(
                        out=xT[:, nt, :],
                        in_=xbf[:, nt * 128:(nt + 1) * 128],
                        transpose=True,
                    )

                for p in ("k", "q", "v"):
                    ptile = pt_pool.tile([128, T], bf16, name=f"pt_{p}_{b}",
                                         tag=f"pt_{p}_{b}")
                    for ic in range(ICH):
                        pps = ps_s.tile([128, IC_LEN], f32, name=f"pps_{p}_{b}_{ic}",
                                        tag="s")
                        for s in range(2):
                            for nt in range(NT):
                                nc.tensor.matmul(
                                    pps[:, s * 512:(s + 1) * 512],
                                    lhsT=wt[p][:, nt, :],
                                    rhs=xT[:, nt,
                                           ic * IC_LEN + s * 512:
                                           ic * IC_LEN + (s + 1) * 512],
                                    start=(nt == 0), stop=(nt == NT - 1),
                                )
                        nc.vector.tensor_scalar_add(
                            out=ptile[:, ic * IC_LEN:(ic + 1) * IC_LEN],
                            in0=pps[:],
                            scalar1=bias_t[p][:],
                        )
                    pt[(p, b)] = ptile

                # V natural layout + ones column: transpose V^T with the PE
                for h in range(H_PER_CORE):
                    v_t = vp_pool.tile([128, JT, 65], bf16, name=f"vp_{h}_{b}",
                                       tag=f"vp_{h}_{b}")
                    nc.vector.memset(v_t[:, :, 64:65], 1.0)
                    vp[(h, b)] = v_t
                for grp in range(2):  # noqa: B007
                    vps = ps_av.tile([128, 2, 512], f32, name=f"vps_{b}_{grp}",
                                     tag="av")
                    for j in range(8):
                        tt = grp * 8 + j
                        nc.tensor.matmul(
                            vps[:, j // 4, (j % 4) * 128:(j % 4) * 128 + 128],
                            lhsT=pt[("v", b)][:, tt * 128:(tt + 1) * 128],
                            rhs=identb[:],
                            start=True, stop=True,
                        )
                    for s in range(2):
                        blk = vps[:, s, :].rearrange("p (c m) -> p c m", m=128)
                        for h in range(H_PER_CORE):
                            nc.vector.tensor_copy(
                                out=vp[(h, b)][:, grp * 8 + s * 4:
                                               grp * 8 + s * 4 + 4, 0:64],
                                in_=blk[:, :, h * 64:h * 64 + 64],
                            )

            # --- attention -------------------------------------------------
            out_v = out.rearrange("(ic it p) b (h n) -> ic b h p it n",
                                  it=IT, p=128, h=H_PER_CORE)

            def attn_batch(b):
                for h in range(H_PER_CORE):
                    qv = pt[("q", b)]
                    kv = pt[("k", b)]
                    for ich in range(ICH):
                        # scores + exp for all 16 k-tiles of this i-chunk
                        esl = []
                        for jt in range(JT):
                            sps = ps_s.tile([128, IC_LEN], f32,
                                            name=f"sps_{b}_{h}_{ich}_{jt}", tag="s")
                            for s in range(2):
                                nc.tensor.matmul(
                                    sps[:, s * 512:(s + 1) * 512],
                                    lhsT=qv[h * 64:(h + 1) * 64,
                                            jt * 128:(jt + 1) * 128],
                                    rhs=kv[h * 64:(h + 1) * 64,
                                           ich * IC_LEN + s * 512:
                                           ich * IC_LEN + (s + 1) * 512],
                                    start=True, stop=True,
                                )
                            es = es_pool.tile([128, IC_LEN], bf16,
                                              name=f"es_{b}_{h}_{ich}_{jt}", tag="es")
                            nc.scalar.activation(out=es[:], in_=sps[:], func=AF.Exp,
                                                 scale=1.0 / 32.0)
                            esl.append(es)
                        # A @ [V, 1]: one 1-bank accumulator per i-tile so each
                        # PSUM zero region holds exactly one accumulation group
                        linv = sm_pool.tile([128, 8, 1], f32,
                                            name=f"linv_{b}_{h}_{ich}", tag="linv")
                        outf = of_pool.tile([128, IT, 64], f32,
                                            name=f"outf_{b}_{h}_{ich}", tag="of")
                        for it in range(IT):
                            av = ps_av.tile([128, 65], f32,
                                            name=f"av_{b}_{h}_{ich}_{it}", tag="av")
                            for jt in range(JT):
                                nc.tensor.matmul(
                                    av[:],
                                    lhsT=esl[jt][:, it * 128:(it + 1) * 128],
                                    rhs=vp[(h, b)][:, jt, :],
                                    start=(jt == 0), stop=(jt == JT - 1),
                                )
                            lv = linv[:, it:it + 1, :]
                            nc.vector.reciprocal(out=lv, in_=av[:, 64:65])
                            rep = bass.AP(tensor=lv.tensor, offset=lv.offset,
                                          ap=[lv.ap[0], [0, 64]])
                            nc.vector.tensor_mul(
                                out=outf[:, it, :],
                                in0=av[:, 0:64],
                                in1=rep,
                            )
                        nc.sync.dma_start(out=out_v[ich, b, h], in_=outf[:])

            for b in range(B):
                prep_batch(b)
            for b in range(B):
                attn_batch(b)
    nc.compile()  # bacc passes: regalloc, DCE, act-table loads, ...
    return nc


def _get_nc():
    if "nc" not in _CACHE:
        _CACHE["nc"] = _build()
    return _CACHE["nc"]


def run(inputs, trace=False, trace_kwargs=None):
    """Run on 8 NeuronCores. Returns (full_output, BassKernelResults)."""
    from concourse.bass_utils import run_bass_kernel_spmd

    nc = _get_nc()
    x = np.ascontiguousarray(np.asarray(inputs["x"], dtype=np.float32))
    in_maps = []
    for c in range(NCORES):
        sl = slice(c * GD, (c + 1) * GD)
        in_maps.append({
            "x": x,
            "wk": np.ascontiguousarray(np.asarray(inputs["Wk"], np.float32)[sl]),
            "wq": np.ascontiguousarray(np.asarray(inputs["Wq"], np.float32)[sl]),
            "wv": np.ascontiguousarray(np.asarray(inputs["Wv"], np.float32)[sl]),
            "bk": np.ascontiguousarray(np.asarray(inputs["bk"], np.float32)[sl]),
            "bq": np.ascontiguousarray(np.asarray(inputs["bq"], np.float32)[sl]),
            "bv": np.ascontiguousarray(np.asarray(inputs["bv"], np.float32)[sl]),
        })
    res = run_bass_kernel_spmd(nc, in_maps, core_ids=list(range(NCORES)),
                               trace=trace, **(trace_kwargs or {}))
    outs = [np.asarray(res.results[c]["out"]) for c in range(NCORES)]
    full = np.concatenate(outs, axis=2).astype(np.float32)
    return full, res


def kernel(x, mask, Wk, bk, Wq, bq, Wv, bv):
    """Full (unsharded) inputs -> full (T, B, H*N_V) float32 output.

    mask is all-True for this problem (spec fill: ones) and is ignored.
    """
    full, _ = run(dict(x=x, mask=mask, Wk=Wk, bk=bk, Wq=Wq, bq=bq, Wv=Wv, bv=bv))
    return full



# revision 2
# speedup vs baseline: 1.1861x; 1.1861x over previous
"""Trainium2 Bass kernel: nn_AttentionLayer (T=2048, B=2, H=16, N_in=1024, d=64).

Sharding: head-parallel across 8 NeuronCores. Each core gets the full x plus a
128-row slice (2 heads) of Wk/Wq/Wv and biases, computes attention for its 2
heads x 2 batches, and writes out[:, :, c*128:(c+1)*128]. The host concatenates
the 8 shards along the feature axis. No cross-core collectives.

Per-core math (matching the reference):
  K^T, Q^T, V^T = W_slice @ x_b^T + bias          (out^T layout, [128, T])
  S^T[k, i]     = sum_n Q^T[n,k] * K^T[n,i]        (k = softmax/query axis)
  E             = exp(S^T / 32)                    (bf16, safe range: |S/32| < ~1.5)
  out'[i, 0:64] = sum_k E[k,i] * V[k, :]           (A@V numerator)
  out'[i, 64]   = sum_k E[k,i]                     (softmax denominator via a
                                                    ones-column appended to V)
  out[i, :]     = out'[i, 0:64] / out'[i, 64]

Pipeline design (v2):
  - x^T is produced on-chip: f32 tiles DMA'd in, cast to bf16 on GpSimd,
    transposed via PE identity-matmuls, copied PSUM->SBUF on Vector. This
    removes the serial DRAM-bounce + DMA-xbar-transpose prefix of v1.
  - Projections/scores cascade at 512-row granularity so the Scalar engine
    (the exp, ~147us of ACTIVATE work - the roofline) starts ~10us in.
  - The two heads' score matmuls (contraction=64) are issued adjacently so
    they run concurrently in different PE row-groups (tile_position packing).
  - Chunk pipeline: scores of chunk N+1 interleave with A@V of chunk N in
    the PE queue so the PE never head-of-line blocks on the exp.
  - PSUM: 3x 2-bank rotating slots (scores/transposes/projections) + 2x
    1-bank A@V accumulators.
"""

import numpy as np

T = 2048
B = 2
NIN = 1024
NQK = 64
NCORES = 8
H_PER_CORE = 2
GD = H_PER_CORE * NQK  # 128: per-core projection width (2 heads x 64)

NT = NIN // 128   # 8  n-tiles (contraction tiles for projections)
TT = T // 128     # 16 t-tiles
IC = 4            # i-chunks per batch
IC_LEN = T // IC  # 512
ITC = IC_LEN // 128  # 4 i-tiles per chunk
JT = TT           # 16 k-tiles (softmax axis)

_CACHE = {}


def _build():
    import concourse.bass as bass
    import concourse.tile as tile
    from concourse import bacc, mybir
    from concourse.masks import make_identity

    f32 = mybir.dt.float32
    bf16 = mybir.dt.bfloat16
    AF = mybir.ActivationFunctionType

    nc = bacc.Bacc("TRN2", target_bir_lowering=False, debug=False,
                   num_devices=NCORES)

    x = nc.dram_tensor("x", [T, B, NIN], f32, kind="ExternalInput").ap()
    w_in = {
        "k": nc.dram_tensor("wk", [GD, NIN], f32, kind="ExternalInput").ap(),
        "q": nc.dram_tensor("wq", [GD, NIN], f32, kind="ExternalInput").ap(),
        "v": nc.dram_tensor("wv", [GD, NIN], f32, kind="ExternalInput").ap(),
    }
    b_in = {
        "k": nc.dram_tensor("bk", [GD], f32, kind="ExternalInput").ap(),
        "q": nc.dram_tensor("bq", [GD], f32, kind="ExternalInput").ap(),
        "v": nc.dram_tensor("bv", [GD], f32, kind="ExternalInput").ap(),
    }
    out = nc.dram_tensor("out", [T, B, GD], f32, kind="ExternalOutput").ap()

    with tile.TileContext(nc) as tc:
        with (
            tc.tile_pool(name="const", bufs=1) as const_pool,
            tc.tile_pool(name="wstage", bufs=2) as wstage,
            tc.tile_pool(name="wt", bufs=1) as wt_pool,
            tc.tile_pool(name="xf", bufs=3) as xf_pool,
            tc.tile_pool(name="xb", bufs=3) as xb_pool,
            tc.tile_pool(name="xt", bufs=1) as xt_pool,
            tc.tile_pool(name="pt", bufs=1) as pt_pool,
            tc.tile_pool(name="vp", bufs=1) as vp_pool,
            tc.tile_pool(name="es", bufs=33) as es_pool,
            tc.tile_pool(name="of", bufs=3) as of_pool,
            tc.tile_pool(name="sm", bufs=4) as sm_pool,
            tc.tile_pool(name="ps_s", bufs=3, space="PSUM") as ps_s,
            tc.tile_pool(name="ps_av", bufs=2, space="PSUM") as ps_av,
        ):
            # --- constants -------------------------------------------------
            ident = const_pool.tile([128, 128], f32, name="ident")
            make_identity(nc, ident)
            identb = const_pool.tile([128, 128], bf16, name="identb")
            nc.vector.tensor_copy(out=identb[:], in_=ident[:])

            bias_t = {}
            for p in ("k", "q", "v"):
                bt = const_pool.tile([128, 1], f32, name=f"bias_{p}")
                nc.sync.dma_start(out=bt[:], in_=b_in[p].rearrange("(p o) -> p o", o=1))
                bias_t[p] = bt

            # --- weights: W [128, 1024] -> W^T tiles [128(n), nt, 128(g)] bf16
            wt = {}
            for p in ("k", "q", "v"):
                wnat = wstage.tile([128, NIN], f32, name=f"wnat_{p}", tag="wnat")
                nc.sync.dma_start(out=wnat[:], in_=w_in[p])
                wps = ps_s.tile([128, NT, 128], f32, name=f"wps_{p}", tag="s")
                for nt in range(NT):
                    nc.tensor.matmul(
                        wps[:, nt, :],
                        lhsT=wnat[:, nt * 128:(nt + 1) * 128],
                        rhs=ident[:],
                        start=True, stop=True,
                    )
                w_t = wt_pool.tile([128, NT, 128], bf16, name=f"wt_{p}", tag=f"wt_{p}")
                nc.vector.tensor_copy(out=w_t[:], in_=wps[:])
                wt[p] = w_t

            # --- x ingest: DMA f32 tiles, cast bf16 (gpsimd), PE-transpose
            xT = {}
            for b in range(B):
                xT[b] = xt_pool.tile([128, NT, T], bf16, name=f"xT_{b}",
                                     tag=f"xT_{b}")
            xf_tiles = {}
            for b in range(B):
                for tt in range(TT):
                    xf = xf_pool.tile([128, NIN], f32, name=f"xf_{b}_{tt}",
                                      tag="xf")
                    nc.sync.dma_start(out=xf[:], in_=x[tt * 128:(tt + 1) * 128, b, :])
                    xf_tiles[(b, tt)] = xf

            def xt_tile(b, tt):
                """cast + PE-transpose one 128-row tile of x into xT[b]."""
                xbf = xb_pool.tile([128, NIN], bf16, name=f"xb_{b}_{tt}", tag="xb")
                nc.gpsimd.tensor_copy(out=xbf[:], in_=xf_tiles[(b, tt)][:])
                psx = ps_s.tile([128, NT, 128], f32, name=f"psx_{b}_{tt}", tag="s")
                for nt in range(NT):
                    nc.tensor.matmul(
                        psx[:, nt, :],
                        lhsT=xbf[:, nt * 128:(nt + 1) * 128],
                        rhs=identb[:],
                        start=True, stop=True,
                    )
                nc.vector.tensor_copy(out=xT[b][:, :, tt * 128:(tt + 1) * 128],
                                      in_=psx[:])

            # --- projections ----------------------------------------------
            pt = {}   # pt[(p, b)]: [128(g), T] bf16   (g = 2 heads x 64)
            for b in range(B):
                for p in ("k", "q", "v"):
                    pt[(p, b)] = pt_pool.tile([128, T], bf16, name=f"pt_{p}_{b}",
                                              tag=f"pt_{p}_{b}")

            def proj_block(p, b, g):
                """project i-range [g*512, (g+1)*512) for p in (k,q,v)."""
                pps = ps_s.tile([128, IC_LEN], f32, name=f"pps_{p}_{b}_{g}",
                                tag="s")
                for nt in range(NT):
                    nc.tensor.matmul(
                        pps[:],
                        lhsT=wt[p][:, nt, :],
                        rhs=xT[b][:, nt, g * IC_LEN:(g + 1) * IC_LEN],
                        start=(nt == 0), stop=(nt == NT - 1),
                    )
                nc.vector.tensor_scalar_add(
                    out=pt[(p, b)][:, g * IC_LEN:(g + 1) * IC_LEN],
                    in0=pps[:],
                    scalar1=bias_t[p][:],
                )

            # --- V natural layout + ones column ---------------------------
            vp = {}   # vp[(h, b)]: [128(t), JT, 65] bf16 (V plus ones column)
            for b in range(B):
                for h in range(H_PER_CORE):
                    v_t = vp_pool.tile([128, JT, 65], bf16, name=f"vp_{h}_{b}",
                                       tag=f"vp_{h}_{b}")
                    vp[(h, b)] = v_t

            def vp_memset(b):
                for h in range(H_PER_CORE):
                    nc.vector.memset(vp[(h, b)][:, :, 64:65], 1.0)

            def vT_group(b, grp):
                """transpose V^T t-tiles [8*grp, 8*grp+8) into vp[(h, b)]."""
                vps = ps_s.tile([128, 8, 128], f32, name=f"vps_{b}_{grp}", tag="s")
                for j in range(8):
                    tt = grp * 8 + j
                    nc.tensor.matmul(
                        vps[:, j, :],
                        lhsT=pt[("v", b)][:, tt * 128:(tt + 1) * 128],
                        rhs=identb[:],
                        start=True, stop=True,
                    )
                for h in range(H_PER_CORE):
                    nc.vector.tensor_copy(
                        out=vp[(h, b)][:, grp * 8:grp * 8 + 8, 0:64],
                        in_=vps[:, :, h * 64:h * 64 + 64],
                    )

            # --- attention chunks -----------------------------------------
            # chunk n = (b, ic): i-range [ic*512, (ic+1)*512), both heads.
            out_v = out.rearrange("(ic it p) b (h n) -> ic b h p it n",
                                  it=ITC, p=128, h=H_PER_CORE)
            es_units = {}   # es_units[(chunk, jt)] = [128, 1024] bf16 (h0|h1)

            def score_unit(n, jt):
                """scores+exp for k-tile jt of chunk n, both heads packed."""
                b, ic = divmod(n, IC)
                qv, kv = pt[("q", b)], pt[("k", b)]
                sq = ps_s.tile([128, 2 * IC_LEN], f32, name=f"sq_{n}_{jt}",
                               tag="s")
                for h in range(H_PER_CORE):
                    nc.tensor.matmul(
                        sq[:, h * IC_LEN:(h + 1) * IC_LEN],
                        lhsT=qv[h * 64:(h + 1) * 64, jt * 128:(jt + 1) * 128],
                        rhs=kv[h * 64:(h + 1) * 64,
                               ic * IC_LEN:(ic + 1) * IC_LEN],
                        start=True, stop=True,
                    )
                es = es_pool.tile([128, 2 * IC_LEN], bf16, name=f"es_{n}_{jt}",
                                  tag="es")
                nc.scalar.activation(out=es[:], in_=sq[:], func=AF.Exp,
                                     scale=1.0 / 32.0)
                es_units[(n, jt)] = es

            def av_group(n, g, outf):
                """A@V accumulation for group g = (h, it) of chunk n + norm."""
                b, ic = divmod(n, IC)
                h, it = divmod(g, ITC)
                av = ps_av.tile([128, 65], f32, name=f"av_{n}_{g}", tag="av")
                for jt in range(JT):
                    nc.tensor.matmul(
                        av[:],
                        lhsT=es_units[(n, jt)][:, h * IC_LEN + it * 128:
                                               h * IC_LEN + (it + 1) * 128],
                        rhs=vp[(h, b)][:, jt, :],
                        start=(jt == 0), stop=(jt == JT - 1),
                    )
                lv = sm_pool.tile([128, 1], f32, name=f"lv_{n}_{g}", tag="lv")
                nc.vector.reciprocal(out=lv[:], in_=av[:, 64:65])
                nc.vector.tensor_scalar_mul(
                    out=outf[:, g, :],
                    in0=av[:, 0:64],
                    scalar1=lv[:],
                )

            def out_dma(n, outf):
                b, ic = divmod(n, IC)
                for h in range(H_PER_CORE):
                    nc.sync.dma_start(
                        out=out_v[ic, b, h],
                        in_=outf[:, h * ITC:(h + 1) * ITC, :],
                    )

            # --- issue order ----------------------------------------------
            # Block 0: b0 ingest cascade + chunk-0 scores.
            for g in range(IC):
                for tt in range(4 * g, 4 * g + 4):
                    xt_tile(0, tt)
                proj_block("q", 0, g)
                proj_block("k", 0, g)
                for jt in range(4 * g, 4 * g + 4):
                    score_unit(0, jt)
            for g in range(IC):
                proj_block("v", 0, g)
            vp_memset(0)
            for grp in range(2):
                vT_group(0, grp)

            # b1 prep pieces, interleaved into blocks 1..3 below.
            def b1_prep_piece(n, step):
                if n == 1:
                    xt_tile(1, 2 * step)
                    xt_tile(1, 2 * step + 1)
                elif n == 2:
                    p = "q" if step < 4 else "k"
                    proj_block(p, 1, step % 4)
                elif n == 3:
                    if step < 4:
                        proj_block("v", 1, step)
                    elif step == 4:
                        vp_memset(1)
                        vT_group(1, 0)
                    elif step == 5:
                        vT_group(1, 1)

            # Blocks 1..7: chunk-n scores interleaved with chunk-(n-1) A@V.
            n_chunks = B * IC
            for n in range(1, n_chunks):
                outf = of_pool.tile([128, H_PER_CORE * ITC, 64], f32,
                                    name=f"outf_{n - 1}", tag="of")
                for step in range(8):
                    score_unit(n, 2 * step)
                    score_unit(n, 2 * step + 1)
                    b1_prep_piece(n, step)
                    av_group(n - 1, step, outf)
                out_dma(n - 1, outf)
            # Tail: last chunk's A@V.
            outf = of_pool.tile([128, H_PER_CORE * ITC, 64], f32,
                                name=f"outf_{n_chunks - 1}", tag="of")
            for step in range(8):
                av_group(n_chunks - 1, step, outf)
            out_dma(n_chunks - 1, outf)

    nc.compile()  # bacc passes: regalloc, DCE, act-table loads, ...
    return nc


def _get_nc():
    if "nc" not in _CACHE:
        _CACHE["nc"] = _build()
    return _CACHE["nc"]


def run(inputs, trace=False, trace_kwargs=None):
    """Run on 8 NeuronCores. Returns (full_output, BassKernelResults)."""
    from concourse.bass_utils import run_bass_kernel_spmd

    nc = _get_nc()
    x = np.ascontiguousarray(np.asarray(inputs["x"], dtype=np.float32))
    in_maps = []
    for c in range(NCORES):
        sl = slice(c * GD, (c + 1) * GD)
        in_maps.append({
            "x": x,
            "wk": np.ascontiguousarray(np.asarray(inputs["Wk"], np.float32)[sl]),
            "wq": np.ascontiguousarray(np.asarray(inputs["Wq"], np.float32)[sl]),
            "wv": np.ascontiguousarray(np.asarray(inputs["Wv"], np.float32)[sl]),
            "bk": np.ascontiguousarray(np.asarray(inputs["bk"], np.float32)[sl]),
            "bq": np.ascontiguousarray(np.asarray(inputs["bq"], np.float32)[sl]),
            "bv": np.ascontiguousarray(np.asarray(inputs["bv"], np.float32)[sl]),
        })
    res = run_bass_kernel_spmd(nc, in_maps, core_ids=list(range(NCORES)),
                               trace=trace, **(trace_kwargs or {}))
    outs = [np.asarray(res.results[c]["out"]) for c in range(NCORES)]
    full = np.concatenate(outs, axis=2).astype(np.float32)
    return full, res


def kernel(x, mask, Wk, bk, Wq, bq, Wv, bv):
    """Full (unsharded) inputs -> full (T, B, H*N_V) float32 output.

    mask is all-True for this problem (spec fill: ones) and is ignored.
    """
    full, _ = run(dict(x=x, mask=mask, Wk=Wk, bk=bk, Wq=Wq, bq=bq, Wv=Wv, bv=bv))
    return full


# revision 3
# speedup vs baseline: 1.2364x; 1.0424x over previous
"""Trainium2 Bass kernel: nn_AttentionLayer (T=2048, B=2, H=16, N_in=1024, d=64).

Sharding: head-parallel across 8 NeuronCores. Each core gets the full x plus a
128-row slice (2 heads) of Wk/Wq/Wv and biases, computes attention for its 2
heads x 2 batches, and writes out[:, :, c*128:(c+1)*128]. The host concatenates
the 8 shards along the feature axis. No cross-core collectives.

Per-core math (matching the reference):
  K^T, Q^T, V^T = W_slice @ x_b^T + bias          (out^T layout, [128, T])
  S^T[k, i]     = sum_n Q^T[n,k] * K^T[n,i]        (k = softmax/query axis)
  E             = exp(S^T / 32)                    (bf16, safe range: |S/32| < ~1.5)
  out'[i, 0:64] = sum_k E[k,i] * V[k, :]           (A@V numerator)
  out'[i, 64]   = sum_k E[k,i]                     (softmax denominator via a
                                                    ones-column appended to V)
  out[i, :]     = out'[i, 0:64] / out'[i, 64]

Pipeline design (v2):
  - x^T is produced on-chip: f32 tiles DMA'd in, cast to bf16 on GpSimd,
    transposed via PE identity-matmuls, copied PSUM->SBUF on Vector. This
    removes the serial DRAM-bounce + DMA-xbar-transpose prefix of v1.
  - Projections/scores cascade at 512-row granularity so the Scalar engine
    (the exp, ~147us of ACTIVATE work - the roofline) starts ~10us in.
  - The two heads' score matmuls (contraction=64) are issued adjacently so
    they run concurrently in different PE row-groups (tile_position packing).
  - Chunk pipeline: scores of chunk N+1 interleave with A@V of chunk N in
    the PE queue so the PE never head-of-line blocks on the exp.
  - PSUM: 3x 2-bank rotating slots (scores/transposes/projections) + 2x
    1-bank A@V accumulators.
"""

import numpy as np

T = 2048
B = 2
NIN = 1024
NQK = 64
NCORES = 8
H_PER_CORE = 2
GD = H_PER_CORE * NQK  # 128: per-core projection width (2 heads x 64)

NT = NIN // 128   # 8  n-tiles (contraction tiles for projections)
TT = T // 128     # 16 t-tiles
IC = 4            # i-chunks per batch
IC_LEN = T // IC  # 512
ITC = IC_LEN // 128  # 4 i-tiles per chunk
JT = TT           # 16 k-tiles (softmax axis)

_CACHE = {}


def _build():
    import concourse.bass as bass
    import concourse.tile as tile
    from concourse import bacc, mybir
    from concourse.masks import make_identity

    f32 = mybir.dt.float32
    bf16 = mybir.dt.bfloat16
    AF = mybir.ActivationFunctionType

    nc = bacc.Bacc("TRN2", target_bir_lowering=False, debug=False,
                   num_devices=NCORES)

    x = nc.dram_tensor("x", [T, B, NIN], f32, kind="ExternalInput").ap()
    w_in = {
        "k": nc.dram_tensor("wk", [GD, NIN], f32, kind="ExternalInput").ap(),
        "q": nc.dram_tensor("wq", [GD, NIN], f32, kind="ExternalInput").ap(),
        "v": nc.dram_tensor("wv", [GD, NIN], f32, kind="ExternalInput").ap(),
    }
    b_in = {
        "k": nc.dram_tensor("bk", [GD], f32, kind="ExternalInput").ap(),
        "q": nc.dram_tensor("bq", [GD], f32, kind="ExternalInput").ap(),
        "v": nc.dram_tensor("bv", [GD], f32, kind="ExternalInput").ap(),
    }
    out = nc.dram_tensor("out", [T, B, GD], f32, kind="ExternalOutput").ap()

    with tile.TileContext(nc) as tc:
        with (
            tc.tile_pool(name="const", bufs=1) as const_pool,
            tc.tile_pool(name="wstage", bufs=2) as wstage,
            tc.tile_pool(name="wt", bufs=1) as wt_pool,
            tc.tile_pool(name="xf", bufs=3) as xf_pool,
            tc.tile_pool(name="xb", bufs=3) as xb_pool,
            tc.tile_pool(name="xt", bufs=1) as xt_pool,
            tc.tile_pool(name="pt", bufs=1) as pt_pool,
            tc.tile_pool(name="vp", bufs=1) as vp_pool,
            tc.tile_pool(name="es", bufs=33) as es_pool,
            tc.tile_pool(name="of", bufs=3) as of_pool,
            tc.tile_pool(name="sm", bufs=4) as sm_pool,
            tc.tile_pool(name="ps_s", bufs=3, space="PSUM") as ps_s,
            tc.tile_pool(name="ps_av", bufs=2, space="PSUM") as ps_av,
        ):
            # --- constants -------------------------------------------------
            ident = const_pool.tile([128, 128], f32, name="ident")
            make_identity(nc, ident)
            identb = const_pool.tile([128, 128], bf16, name="identb")
            nc.vector.tensor_copy(out=identb[:], in_=ident[:])

            # x f32 tile DMAs for the first cascade group go out first so the
            # PE can start transposing before the (larger) W transfer lands.
            xT = {}
            for b in range(B):
                xT[b] = xt_pool.tile([128, NT, T], bf16, name=f"xT_{b}",
                                     tag=f"xT_{b}")
            xf_tiles = {}

            def xf_dma(b, tt):
                xf = xf_pool.tile([128, NIN], f32, name=f"xf_{b}_{tt}",
                                  tag="xf")
                nc.sync.dma_start(out=xf[:], in_=x[tt * 128:(tt + 1) * 128, b, :])
                xf_tiles[(b, tt)] = xf

            for tt in range(4):
                xf_dma(0, tt)

            bias_t = {}
            for p in ("k", "q", "v"):
                bt = const_pool.tile([128, 1], f32, name=f"bias_{p}")
                nc.sync.dma_start(out=bt[:], in_=b_in[p].rearrange("(p o) -> p o", o=1))
                bias_t[p] = bt

            # --- weights: W [128, 1024] -> W^T tiles [128(n), nt, 128(g)] bf16
            wt = {}
            for p in ("k", "q", "v"):
                wnat = wstage.tile([128, NIN], f32, name=f"wnat_{p}", tag="wnat")
                nc.sync.dma_start(out=wnat[:], in_=w_in[p])
                wbf = wstage.tile([128, NIN], bf16, name=f"wbf_{p}", tag="wbf")
                nc.vector.tensor_copy(out=wbf[:], in_=wnat[:])
                wps = ps_s.tile([128, NT, 128], f32, name=f"wps_{p}", tag="s")
                for nt in range(NT):
                    nc.tensor.matmul(
                        wps[:, nt, :],
                        lhsT=wbf[:, nt * 128:(nt + 1) * 128],
                        rhs=identb[:],
                        start=True, stop=True,
                    )
                w_t = wt_pool.tile([128, NT, 128], bf16, name=f"wt_{p}", tag=f"wt_{p}")
                nc.vector.tensor_copy(out=w_t[:], in_=wps[:])
                wt[p] = w_t

            for tt in range(4, TT):
                xf_dma(0, tt)
            for tt in range(TT):
                xf_dma(1, tt)

            def xt_tile(b, tt):
                """cast + PE-transpose one 128-row tile of x into xT[b]."""
                xbf = xb_pool.tile([128, NIN], bf16, name=f"xb_{b}_{tt}", tag="xb")
                nc.vector.tensor_copy(out=xbf[:], in_=xf_tiles[(b, tt)][:])
                psx = ps_s.tile([128, NT, 128], f32, name=f"psx_{b}_{tt}", tag="s")
                for nt in range(NT):
                    nc.tensor.matmul(
                        psx[:, nt, :],
                        lhsT=xbf[:, nt * 128:(nt + 1) * 128],
                        rhs=identb[:],
                        start=True, stop=True,
                    )
                nc.vector.tensor_copy(out=xT[b][:, :, tt * 128:(tt + 1) * 128],
                                      in_=psx[:])

            # --- projections ----------------------------------------------
            pt = {}   # pt[(p, b)]: [128(g), T] bf16   (g = 2 heads x 64)
            for b in range(B):
                for p in ("k", "q", "v"):
                    pt[(p, b)] = pt_pool.tile([128, T], bf16, name=f"pt_{p}_{b}",
                                              tag=f"pt_{p}_{b}")

            def proj_block(p, b, g):
                """project i-range [g*512, (g+1)*512) for p in (k,q,v)."""
                pps = ps_s.tile([128, IC_LEN], f32, name=f"pps_{p}_{b}_{g}",
                                tag="s")
                for nt in range(NT):
                    nc.tensor.matmul(
                        pps[:],
                        lhsT=wt[p][:, nt, :],
                        rhs=xT[b][:, nt, g * IC_LEN:(g + 1) * IC_LEN],
                        start=(nt == 0), stop=(nt == NT - 1),
                    )
                nc.vector.tensor_scalar_add(
                    out=pt[(p, b)][:, g * IC_LEN:(g + 1) * IC_LEN],
                    in0=pps[:],
                    scalar1=bias_t[p][:],
                )

            # --- V natural layout + ones column ---------------------------
            vp = {}   # vp[(h, b)]: [128(t), JT, 65] bf16 (V plus ones column)
            for b in range(B):
                for h in range(H_PER_CORE):
                    v_t = vp_pool.tile([128, JT, 65], bf16, name=f"vp_{h}_{b}",
                                       tag=f"vp_{h}_{b}")
                    vp[(h, b)] = v_t

            def vp_memset(b):
                for h in range(H_PER_CORE):
                    nc.vector.memset(vp[(h, b)][:, :, 64:65], 1.0)

            def vT_group(b, grp):
                """transpose V^T t-tiles [8*grp, 8*grp+8) into vp[(h, b)]."""
                vps = ps_s.tile([128, 8, 128], f32, name=f"vps_{b}_{grp}", tag="s")
                for j in range(8):
                    tt = grp * 8 + j
                    nc.tensor.matmul(
                        vps[:, j, :],
                        lhsT=pt[("v", b)][:, tt * 128:(tt + 1) * 128],
                        rhs=identb[:],
                        start=True, stop=True,
                    )
                for h in range(H_PER_CORE):
                    nc.vector.tensor_copy(
                        out=vp[(h, b)][:, grp * 8:grp * 8 + 8, 0:64],
                        in_=vps[:, :, h * 64:h * 64 + 64],
                    )

            # --- attention chunks -----------------------------------------
            # chunk n = (b, ic): i-range [ic*512, (ic+1)*512), both heads.
            out_v = out.rearrange("(ic it p) b (h n) -> ic b h p it n",
                                  it=ITC, p=128, h=H_PER_CORE)
            es_units = {}   # es_units[(chunk, jt)] = [128, 1024] bf16 (h0|h1)

            def score_unit(n, jt):
                """scores+exp for k-tile jt of chunk n, both heads packed."""
                b, ic = divmod(n, IC)
                qv, kv = pt[("q", b)], pt[("k", b)]
                sq = ps_s.tile([128, 2 * IC_LEN], f32, name=f"sq_{n}_{jt}",
                               tag="s")
                for h in range(H_PER_CORE):
                    nc.tensor.matmul(
                        sq[:, h * IC_LEN:(h + 1) * IC_LEN],
                        lhsT=qv[h * 64:(h + 1) * 64, jt * 128:(jt + 1) * 128],
                        rhs=kv[h * 64:(h + 1) * 64,
                               ic * IC_LEN:(ic + 1) * IC_LEN],
                        start=True, stop=True,
                    )
                es = es_pool.tile([128, 2 * IC_LEN], bf16, name=f"es_{n}_{jt}",
                                  tag="es")
                nc.scalar.activation(out=es[:], in_=sq[:], func=AF.Exp,
                                     scale=1.0 / 32.0)
                es_units[(n, jt)] = es

            def av_group(n, g, outf):
                """A@V accumulation for group g = (h, it) of chunk n + norm."""
                b, ic = divmod(n, IC)
                h, it = divmod(g, ITC)
                av = ps_av.tile([128, 65], f32, name=f"av_{n}_{g}", tag="av")
                for jt in range(JT):
                    nc.tensor.matmul(
                        av[:],
                        lhsT=es_units[(n, jt)][:, h * IC_LEN + it * 128:
                                               h * IC_LEN + (it + 1) * 128],
                        rhs=vp[(h, b)][:, jt, :],
                        start=(jt == 0), stop=(jt == JT - 1),
                    )
                lv = sm_pool.tile([128, 1], f32, name=f"lv_{n}_{g}", tag="lv")
                nc.vector.reciprocal(out=lv[:], in_=av[:, 64:65])
                nc.vector.tensor_scalar_mul(
                    out=outf[:, g, :],
                    in0=av[:, 0:64],
                    scalar1=lv[:],
                )

            def out_dma(n, outf):
                b, ic = divmod(n, IC)
                for h in range(H_PER_CORE):
                    nc.sync.dma_start(
                        out=out_v[ic, b, h],
                        in_=outf[:, h * ITC:(h + 1) * ITC, :],
                    )

            # --- issue order ----------------------------------------------
            # Block 0: b0 ingest cascade + chunk-0 scores.
            for g in range(IC):
                for tt in range(4 * g, 4 * g + 4):
                    xt_tile(0, tt)
                proj_block("q", 0, g)
                proj_block("k", 0, g)
                for jt in range(4 * g, 4 * g + 4):
                    score_unit(0, jt)
            for g in range(IC):
                proj_block("v", 0, g)
            vp_memset(0)
            for grp in range(2):
                vT_group(0, grp)

            # b1 prep pieces, interleaved into blocks 1..3 below.
            def b1_prep_piece(n, step):
                if n == 1:
                    xt_tile(1, 2 * step)
                    xt_tile(1, 2 * step + 1)
                elif n == 2:
                    p = "q" if step < 4 else "k"
                    proj_block(p, 1, step % 4)
                elif n == 3:
                    if step < 4:
                        proj_block("v", 1, step)
                    elif step == 4:
                        vp_memset(1)
                        vT_group(1, 0)
                    elif step == 5:
                        vT_group(1, 1)

            # Blocks 1..7: chunk-n scores interleaved with chunk-(n-1) A@V.
            n_chunks = B * IC
            for n in range(1, n_chunks):
                outf = of_pool.tile([128, H_PER_CORE * ITC, 64], f32,
                                    name=f"outf_{n - 1}", tag="of")
                for step in range(8):
                    score_unit(n, 2 * step)
                    score_unit(n, 2 * step + 1)
                    b1_prep_piece(n, step)
                    av_group(n - 1, step, outf)
                out_dma(n - 1, outf)
            # Tail: last chunk's A@V.
            outf = of_pool.tile([128, H_PER_CORE * ITC, 64], f32,
                                name=f"outf_{n_chunks - 1}", tag="of")
            for step in range(8):
                av_group(n_chunks - 1, step, outf)
            out_dma(n_chunks - 1, outf)

    nc.compile()  # bacc passes: regalloc, DCE, act-table loads, ...
    return nc


def _get_nc():
    if "nc" not in _CACHE:
        _CACHE["nc"] = _build()
    return _CACHE["nc"]


def run(inputs, trace=False, trace_kwargs=None):
    """Run on 8 NeuronCores. Returns (full_output, BassKernelResults)."""
    from concourse.bass_utils import run_bass_kernel_spmd

    nc = _get_nc()
    x = np.ascontiguousarray(np.asarray(inputs["x"], dtype=np.float32))
    in_maps = []
    for c in range(NCORES):
        sl = slice(c * GD, (c + 1) * GD)
        in_maps.append({
            "x": x,
            "wk": np.ascontiguousarray(np.asarray(inputs["Wk"], np.float32)[sl]),
            "wq": np.ascontiguousarray(np.asarray(inputs["Wq"], np.float32)[sl]),
            "wv": np.ascontiguousarray(np.asarray(inputs["Wv"], np.float32)[sl]),
            "bk": np.ascontiguousarray(np.asarray(inputs["bk"], np.float32)[sl]),
            "bq": np.ascontiguousarray(np.asarray(inputs["bq"], np.float32)[sl]),
            "bv": np.ascontiguousarray(np.asarray(inputs["bv"], np.float32)[sl]),
        })
    res = run_bass_kernel_spmd(nc, in_maps, core_ids=list(range(NCORES)),
                               trace=trace, **(trace_kwargs or {}))
    outs = [np.asarray(res.results[c]["out"]) for c in range(NCORES)]
    full = np.concatenate(outs, axis=2).astype(np.float32)
    return full, res


def kernel(x, mask, Wk, bk, Wq, bq, Wv, bv):
    """Full (unsharded) inputs -> full (T, B, H*N_V) float32 output.

    mask is all-True for this problem (spec fill: ones) and is ignored.
    """
    full, _ = run(dict(x=x, mask=mask, Wk=Wk, bk=bk, Wq=Wq, bq=bq, Wv=Wv, bv=bv))
    return full
